# revision 1
# baseline (speedup 1.0000x reference)
# Trainium2 Bass kernel for nn_AttentionStream (dense transformer block with
# relative-position attention), SPMD over 8 NeuronCores.
#
# Sharding: core c -> batch b = c//2, head-group hg = c%2 (4 heads each).
# Each core computes a row-parallel partial of the output projection for its
# batch; the host sums the two partials per batch and adds the bias.
#
# Device algorithm (per core), in "transposed flash" layout so the PV matmul
# needs no transposes:
#   qT/kT = projections of x (d on partitions), v in [r, d] layout
#   epos[n, j] = exp(q~ . relF[j])   (relF host-prepped: reversed, minus
#       rel[dist=far-past] so left-clamp factor == 1, edge-padded)  -> DRAM
#   per (head, n-window): dots^T[r, n] tiles -> exp -> P
#       band tiles:   P *= epos skew-read from DRAM via merged transpose-DMA
#       right tiles:  P *= edelta(n) (host-precomputed per-n factor)
#   PV: acc[65, n] += [v | 1].T @ P   (ones row accumulates the denominator)
#   avn = acc[0:64] / acc[64] ; out^T += Wo_h.T @ avn
import os
import sys

import numpy as np
import ml_dtypes

for _p in ("/opt/trn_rl_repo", "/root/.axon_site/_ro/trn_rl_repo"):
    if _p not in sys.path and os.path.isdir(_p):
        sys.path.append(_p)

B, N, DIM = 4, 2048, 512
H, D = 8, 64          # total heads, head dim
HPC = 4               # heads per core
INNER = H * D
MAXP = 512
SCALE = D ** -0.5
NCORES = 8
W = 1280              # padded j width; j' = j + PAD_L, j in [-128, 1151]
PAD_L = 128
NW = 8                # n-windows of 256
NRC = 16              # r-chunks of 128

BF = ml_dtypes.bfloat16

_CACHE = {}


def _build_bass(debug_dumps=False):
    import concourse.bass as bass
    import concourse.mybir as mybir
    import concourse.tile as tile
    from concourse import bacc

    dt = mybir.dt
    fp32 = dt.float32
    bf16 = dt.bfloat16
    EXP = mybir.ActivationFunctionType.Exp

    nc = bacc.Bacc("TRN2", target_bir_lowering=False, debug=False,
                   num_devices=NCORES)

    dbg = {}
    if debug_dumps:
        dbg["qT"] = nc.dram_tensor("dbg_qT", [128, 2, N], bf16, kind="ExternalOutput")
        dbg["kT"] = nc.dram_tensor("dbg_kT", [128, 2, N], bf16, kind="ExternalOutput")
        dbg["v"] = nc.dram_tensor("dbg_v", [128, NRC, HPC, 65], bf16, kind="ExternalOutput")
        dbg["epos0"] = nc.dram_tensor("dbg_epos0", [N, W], bf16, kind="ExternalOutput")
        dbg["P"] = nc.dram_tensor("dbg_P", [128, NRC, 256], bf16, kind="ExternalOutput")
        dbg["ep"] = nc.dram_tensor("dbg_ep", [128, NRC, 9, 128], bf16, kind="ExternalOutput")
        dbg["num"] = nc.dram_tensor("dbg_num", [65, NW, 256], fp32, kind="ExternalOutput")
        dbg["avn"] = nc.dram_tensor("dbg_avn", [64, HPC, NW, 256], bf16, kind="ExternalOutput")

    xT = nc.dram_tensor("xT", [DIM, N], bf16, kind="ExternalInput")
    wq = nc.dram_tensor("wq", [DIM, 256], bf16, kind="ExternalInput")
    wk = nc.dram_tensor("wk", [DIM, 256], bf16, kind="ExternalInput")
    wv = nc.dram_tensor("wv", [DIM, 256], bf16, kind="ExternalInput")
    relT = nc.dram_tensor("relT", [128, W], bf16, kind="ExternalInput")
    wo = nc.dram_tensor("wo", [256, DIM], bf16, kind="ExternalInput")
    edel = nc.dram_tensor("edel", [HPC, N], bf16, kind="ExternalInput")
    outT = nc.dram_tensor("outT", [DIM, N], fp32, kind="ExternalOutput")

    from contextlib import ExitStack
    with tile.TileContext(nc) as tc, ExitStack() as ctx:
        consts = ctx.enter_context(tc.tile_pool(name="consts", bufs=1))
        work = ctx.enter_context(tc.tile_pool(name="work", bufs=3))
        ppool = ctx.enter_context(tc.tile_pool(name="ppool", bufs=2))
        eppool = ctx.enter_context(tc.tile_pool(name="eppool", bufs=2))
        numep = ctx.enter_context(tc.tile_pool(name="numep", bufs=2))
        psb = ctx.enter_context(tc.tile_pool(name="psb", bufs=3, space="PSUM"))
        psa = ctx.enter_context(tc.tile_pool(name="psa", bufs=2, space="PSUM"))
        dramp = ctx.enter_context(tc.tile_pool(name="dramp", bufs=4, space="DRAM"))

        # ---- load constants (scalar-engine HWDGE queue) ---------------------
        xT_sb = consts.tile([128, 4, N], bf16, tag="xT_sb")
        nc.scalar.dma_start(out=xT_sb, in_=xT.ap().rearrange("(c p) n -> p c n", p=128))
        wq_sb = consts.tile([128, 4, 256], bf16, tag="wq_sb")
        nc.scalar.dma_start(out=wq_sb, in_=wq.ap().rearrange("(c p) i -> p c i", p=128))
        wk_sb = consts.tile([128, 4, 256], bf16, tag="wk_sb")
        nc.scalar.dma_start(out=wk_sb, in_=wk.ap().rearrange("(c p) i -> p c i", p=128))
        wv_sb = consts.tile([128, 4, 256], bf16, tag="wv_sb")
        nc.scalar.dma_start(out=wv_sb, in_=wv.ap().rearrange("(c p) i -> p c i", p=128))
        relT_sb = consts.tile([128, W], bf16, tag="relT_sb")
        nc.scalar.dma_start(out=relT_sb, in_=relT.ap())
        wo_sb = consts.tile([64, HPC, DIM], bf16, tag="wo_sb")
        nc.scalar.dma_start(out=wo_sb, in_=wo.ap().rearrange("(h p) o -> p h o", p=64))
        # edel replicated across all 128 partitions
        edel_sb = consts.tile([128, HPC, N], bf16, tag="edel_sb")
        edel_src = bass.AP(tensor=edel.ap().tensor, offset=edel.ap().offset,
                           ap=[[0, 128], [N, HPC], [1, N]])
        nc.scalar.dma_start(out=edel_sb, in_=edel_src)

        # ---- projections ----------------------------------------------------
        qT_sb = consts.tile([128, 2, N], bf16, tag="qT_sb")
        kT_sb = consts.tile([128, 2, N], bf16, tag="kT_sb")
        for dst_sb, w_sb in ((qT_sb, wq_sb), (kT_sb, wk_sb)):
            for ic in range(2):
                for nw in range(4):   # 512-wide windows
                    ps = psb.tile([128, 1024], fp32, tag="big")
                    for dc in range(4):
                        nc.tensor.matmul(
                            ps[:, 0:512],
                            lhsT=w_sb[:, dc, ic * 128:(ic + 1) * 128],
                            rhs=xT_sb[:, dc, nw * 512:(nw + 1) * 512],
                            start=(dc == 0), stop=(dc == 3))
                    nc.vector.tensor_copy(dst_sb[:, ic, nw * 512:(nw + 1) * 512],
                                          ps[:, 0:512])
        # v in [r, d-per-head] layout with a ones column per head: [128, rc, h, 65]
        v_sb = consts.tile([128, NRC, HPC, 65], bf16, tag="v_sb")
        nc.vector.memset(v_sb[:, :, :, 64], 1.0)
        for rc in range(NRC):
            ps = psb.tile([128, 1024], fp32, tag="big")
            for dc in range(4):
                nc.tensor.matmul(
                    ps[:, 0:256],
                    lhsT=xT_sb[:, dc, rc * 128:(rc + 1) * 128],
                    rhs=wv_sb[:, dc, :],
                    start=(dc == 0), stop=(dc == 3))
            nc.vector.tensor_copy(
                v_sb[:, rc, :, 0:64],
                ps[:, 0:256].rearrange("p (h d) -> p h d", h=HPC))

        if debug_dumps:
            nc.sync.dma_start(out=dbg["qT"].ap(), in_=qT_sb)
            nc.sync.dma_start(out=dbg["kT"].ap(), in_=kT_sb)
            nc.sync.dma_start(out=dbg["v"].ap(), in_=v_sb)

        # ---- per-head phases, emitted interleaved for pipelining ------------
        JW = [(0, 512), (512, 512), (1024, 256)]
        epos_h = [None] * HPC

        def emit_epos(h):
            hc, hp = h // 2, (h % 2) * 64
            ep_dram = dramp.tile([N, W], bf16, tag="epos")
            epos_h[h] = ep_dram
            for nck in range(NRC):
                for (j0, jl) in JW:
                    ps = psb.tile([128, 1024], fp32, tag="big")
                    nc.tensor.matmul(
                        ps[:, 0:jl],
                        lhsT=qT_sb[hp:hp + 64, hc, nck * 128:(nck + 1) * 128],
                        rhs=relT_sb[hp:hp + 64, j0:j0 + jl],
                        start=True, stop=True)
                    et = work.tile([128, 512], bf16, tag="eptile")
                    nc.scalar.activation(et[:, 0:jl], ps[:, 0:jl], EXP)
                    # scalar-engine DMA: depends only on ACT's own output
                    nc.scalar.dma_start(
                        out=ep_dram[nck * 128:(nck + 1) * 128, j0:j0 + jl],
                        in_=et[:, 0:jl])
            if debug_dumps and h == 0:
                nc.sync.dma_start(out=dbg["epos0"].ap(), in_=ep_dram[:, :])

        avn_all = consts.tile([64, HPC, NW, 256], bf16, tag="avn_all")

        def emit_att(h):
            hc, hp = h // 2, (h % 2) * 64
            ep_dram = epos_h[h]
            num_all = numep.tile([65, NW, 256], fp32, tag="num_all")
            den_all = numep.tile([8, 256], fp32, tag="den_all")
            recip_all = numep.tile([8, 256], fp32, tag="recip_all")
            recip_dram = dramp.tile([8, 256], fp32, tag="recip_dram")

            # merged skew transpose-DMAs: one per r-chunk, covering all its
            # band n-subchunks (diagonals d = rc - s in [-4, 4])
            ep_all = eppool.tile([128, NRC, 9, 128], bf16, tag="ep_all")
            for rc in range(NRC):
                s_lo, s_hi = max(0, rc - 4), min(NRC - 1, rc + 4)
                k = s_hi - s_lo + 1
                # src element (s, a, c) = epos[128*s + c, PAD + 128*(rc-s) + 512 + a - c]
                off = (ep_dram.offset + 128 * s_lo * W
                       + PAD_L + 128 * (rc - s_lo) + 512)
                src = bass.AP(tensor=ep_dram.tensor, offset=off,
                              ap=[[W - 1, 128 * k], [1, 128]])
                slot0 = s_lo - (rc - 4)
                nc.sync.dma_start(out=ep_all[:, rc, slot0:slot0 + k, :],
                                  in_=src, transpose=True)

            for nw in range(NW):
                n0 = nw * 256
                s0 = 2 * nw                     # first n-sub of this window
                # dots^T in groups of 4 r-chunks, exp to P
                P_sb = ppool.tile([128, NRC, 256], bf16, tag="P_sb")
                for g in range(4):
                    ps = psb.tile([128, 1024], fp32, tag="big")
                    for i in range(4):
                        rc = 4 * g + i
                        nc.tensor.matmul(
                            ps[:, i * 256:(i + 1) * 256],
                            lhsT=kT_sb[hp:hp + 64, hc, rc * 128:(rc + 1) * 128],
                            rhs=qT_sb[hp:hp + 64, hc, n0:n0 + 256],
                            start=True, stop=True)
                    nc.scalar.activation(
                        P_sb[:, 4 * g:4 * (g + 1), :],
                        ps.rearrange("p (i n) -> p i n", i=4), EXP)

                # band multiplies.  For r-chunk rc and n-sub s: slot = s-(rc-4).
                # Central run rc in [s0-3, s0+4]: both halves in band; the ep
                # slot pair starts at slot0(rc) = s0-rc+4, linear in rc with
                # step (9*128 - 128) in ep_all's free space -> one strided op.
                ca, cb = max(0, s0 - 3), min(NRC - 1, s0 + 4)
                if cb >= ca:
                    kk = cb - ca + 1
                    epa = ep_all[:, ca, s0 - ca + 4, 0:128]   # first element
                    ep_run = bass.AP(
                        tensor=epa.tensor, offset=epa.offset,
                        ap=[list(epa.ap[0]), [8 * 128, kk], [1, 256]])
                    nc.vector.tensor_mul(P_sb[:, ca:cb + 1, :],
                                         P_sb[:, ca:cb + 1, :], ep_run)
                # edge rc = s0-4: half0 (s=s0, rc-s=-4) is band -> ep slot 8;
                # half1 (s=s0+1, rc-s=-5) is left-clamp -> factor 1, skip.
                rc = s0 - 4
                if rc >= 0:
                    nc.vector.tensor_mul(
                        P_sb[:, rc, 0:128], P_sb[:, rc, 0:128],
                        ep_all[:, rc, 8, :])
                # edge rc = s0+5: half0 (s=s0, d=5) right-clamp -> edelta;
                # half1 (s=s0+1, d=4) -> ep slot 0.
                rc = s0 + 5
                if rc <= NRC - 1:
                    nc.vector.tensor_mul(
                        P_sb[:, rc, 0:128], P_sb[:, rc, 0:128],
                        edel_sb[:, h, n0:n0 + 128])
                    nc.vector.tensor_mul(
                        P_sb[:, rc, 128:256], P_sb[:, rc, 128:256],
                        ep_all[:, rc, 0, :])
                # right-of-band: rc >= s0+6 -> edelta on both halves
                rstart = s0 + 6
                if rstart < NRC:
                    k_right = NRC - rstart
                    ed = edel_sb[:, h, n0:n0 + 256]
                    ed_b = bass.AP(tensor=ed.tensor, offset=ed.offset,
                                   ap=[list(ed.ap[0]), [0, k_right], [1, 256]])
                    nc.vector.tensor_mul(
                        P_sb[:, rstart:NRC, :], P_sb[:, rstart:NRC, :], ed_b)

                if debug_dumps and h == 0 and nw == 2:
                    nc.sync.dma_start(out=dbg["ep"].ap(), in_=ep_all)
                    nc.sync.dma_start(out=dbg["P"].ap(), in_=P_sb)

                # PV accumulate: acc[0:65, n] += [v|1].T @ P
                acc = psa.tile([128, 256], fp32, tag="acc")
                for rc in range(NRC):
                    nc.tensor.matmul(
                        acc[0:65, :],
                        lhsT=v_sb[:, rc, h, :],
                        rhs=P_sb[:, rc, :],
                        start=(rc == 0), stop=(rc == NRC - 1),
                        skip_group_check=True)
                nc.vector.tensor_copy(num_all[:, nw, :], acc[0:65, :])
                # collect denominator row for batched reciprocal
                nc.gpsimd.dma_start(out=den_all[nw:nw + 1, :],
                                    in_=num_all[64:65, nw, :])

            if debug_dumps and h == 0:
                nc.sync.dma_start(out=dbg["num"].ap(), in_=num_all)
            nc.vector.reciprocal(recip_all, den_all)
            nc.gpsimd.dma_start(out=recip_dram, in_=recip_all)
            for nw in range(NW):
                rb_bc = work.tile([64, 256], fp32, tag="rb_bc")
                rsrc = bass.AP(tensor=recip_dram.tensor,
                               offset=recip_dram.offset + nw * 256,
                               ap=[[0, 64], [1, 256]])
                nc.gpsimd.dma_start(out=rb_bc, in_=rsrc)
                nc.vector.tensor_mul(avn_all[:, h, nw, :],
                                     num_all[0:64, nw, :], rb_bc)

        # pipeline heads: epos(0), epos(1), att(0), epos(2), att(1), ...
        emit_epos(0)
        emit_epos(1)
        emit_att(0)
        emit_epos(2)
        emit_att(1)
        emit_epos(3)
        emit_att(2)
        emit_att(3)

        if debug_dumps:
            nc.sync.dma_start(out=dbg["avn"].ap(), in_=avn_all)

        # ---- output projection ---------------------------------------------
        for nw in range(NW):
            for oc in range(4):
                ps = psb.tile([128, 1024], fp32, tag="big")
                for h in range(HPC):
                    nc.tensor.matmul(
                        ps[:, 0:256],
                        lhsT=wo_sb[:, h, oc * 128:(oc + 1) * 128],
                        rhs=avn_all[:, h, nw, :],
                        start=(h == 0), stop=(h == HPC - 1))
                o_sb = work.tile([128, 256], fp32, tag="o_sb")
                nc.vector.tensor_copy(o_sb, ps[:, 0:256])
                nc.gpsimd.dma_start(
                    out=outT.ap()[oc * 128:(oc + 1) * 128, nw * 256:(nw + 1) * 256],
                    in_=o_sb)

    nc.compile()
    return nc


def host_prep(x, Wq, Wkv, Wo, bo, rel_emb):
    """Build the 8 per-core input maps (all host-side prep is O(N*D))."""
    x = np.asarray(x, np.float32)
    Wq = np.asarray(Wq, np.float32)
    Wkv = np.asarray(Wkv, np.float32)
    Wo = np.asarray(Wo, np.float32)
    rel_emb = np.asarray(rel_emb, np.float32)

    # relF[j] = rel_emb[1024-j] - rel_emb[1024], edge-padded/clipped; [W, 64]
    jgrid = np.clip(np.arange(W) - PAD_L, 0, 1024)
    relF = rel_emb[1024 - jgrid] - rel_emb[1024]
    relT_one = np.ascontiguousarray(relF.T)            # [64, W]
    relT_in = np.concatenate([relT_one, relT_one], axis=0).astype(BF)  # [128, W]
    d_vec = rel_emb[0] - rel_emb[1024]                 # [64]

    in_maps = []
    for core in range(NCORES):
        b, hg = core // 2, core % 2
        sl = slice(hg * 256, (hg + 1) * 256)
        wq_s = (Wq[:, sl] * SCALE).astype(BF)
        # edelta per head: exp(x @ Wq~_h @ d_vec)
        delta = x[b] @ ((Wq[:, sl] * SCALE).reshape(DIM, HPC, D) @ d_vec)  # [N, HPC]
        in_maps.append({
            "xT": np.ascontiguousarray(x[b].T).astype(BF),
            "wq": wq_s,
            "wk": Wkv[:, sl].astype(BF),
            "wv": Wkv[:, 512 + hg * 256: 512 + (hg + 1) * 256].astype(BF),
            "relT": relT_in,
            "wo": Wo[sl, :].astype(BF),
            "edel": np.ascontiguousarray(np.exp(delta).T).astype(BF),  # [HPC, N]
        })
    return in_maps


def _install_ntff_hook():
    """The agent image's antenv lacks axon_hooks; synthesize it so
    run_bass_kernel_spmd(trace=True) can capture NTFF profiles."""
    import types
    try:
        if "antenv.axon_hooks" not in sys.modules:
            import antenv
            from trn_agent_boot.trn_boot import _ntff_profile_via_ctypes
            hooks = types.ModuleType("antenv.axon_hooks")
            state = {"h": _ntff_profile_via_ctypes("/opt/axon/libaxon_pjrt.so")}
            hooks.set_axon_ntff_profile_hook = lambda h: state.__setitem__("h", h)
            hooks.get_axon_ntff_profile_hook = lambda: state["h"]
            sys.modules["antenv.axon_hooks"] = hooks
            antenv.axon_hooks = hooks
        import antenv.axon_hooks as ah
        return ah.get_axon_ntff_profile_hook() is not None
    except Exception as e:
        print(f"ntff hook install failed: {e!r}")
        return False


def kernel(x, Wq, Wkv, Wo, bo, rel_emb, _trace=False):
    import concourse.bass_utils as bu
    from concourse.bass_utils import run_bass_kernel_spmd

    if "nc" not in _CACHE:
        _CACHE["nc"] = _build_bass()
    nc = _CACHE["nc"]

    in_maps = host_prep(x, Wq, Wkv, Wo, bo, rel_emb)
    kw = {}
    if _trace and _install_ntff_hook():
        bu.upload_artifacts = lambda d: d     # zero-egress: keep artifacts local
        tmpdir = "/root/problem/traces/latest"
        import shutil
        shutil.rmtree(tmpdir, ignore_errors=True)
        os.makedirs(tmpdir, exist_ok=True)
        kw = dict(trace=True, tmpdir=tmpdir)
    res = run_bass_kernel_spmd(nc, in_maps, list(range(NCORES)), **kw)
    _CACHE["last_result"] = res

    bo = np.asarray(bo, np.float32)
    out = np.empty((B, N, DIM), np.float32)
    for b in range(B):
        pT = res.results[2 * b]["outT"] + res.results[2 * b + 1]["outT"]
        out[b] = pT.T + bo[None, :]
    return out



# revision 3
# speedup vs baseline: 1.2118x; 1.2118x over previous
# Trainium2 Bass kernel for nn_AttentionStream (dense transformer block with
# relative-position attention), SPMD over 8 NeuronCores.
#
# Sharding: core c -> batch b = c//2, head-group hg = c%2 (4 heads each).
# Each core computes a row-parallel partial of the output projection for its
# batch; the host sums the two partials per batch and adds the bias.
#
# Device algorithm (per core), "transposed flash" layout (PV needs no
# transposes), with the positional term folded in as a LOGIT ADD:
#   qT/kT = projections of x (d on partitions), v in [r, d] layout
#   L[n, j] = q~ . relF[j]   (relF host-prepped: reversed, minus
#       rel[dist=far-past] so left-clamp add == 0, edge-padded) -> DRAM bf16
#   per (head, n-window): dots^T[r, n] tiles in PSUM
#       band tiles:   PSUM += L skew-read from DRAM via merged transpose-DMA
#       exp (single ACT pass) -> P bf16
#       right tiles:  P *= edelta(n) (host-precomputed per-n exp factor)
#   PV: acc[65, n] += [v | 1].T @ P   (ones row accumulates the denominator)
#   avn = acc[0:64] / acc[64] ; out^T += Wo_hpair.T @ avn (head-pair packed)
import os
import sys

import numpy as np
import ml_dtypes

for _p in ("/opt/trn_rl_repo", "/root/.axon_site/_ro/trn_rl_repo"):
    if _p not in sys.path and os.path.isdir(_p):
        sys.path.append(_p)

B, N, DIM = 4, 2048, 512
H, D = 8, 64          # total heads, head dim
HPC = 4               # heads per core
INNER = H * D
MAXP = 512
SCALE = D ** -0.5
NCORES = 8
W = 1280              # padded j width; j' = j + PAD_L, j in [-128, 1151]
PAD_L = 128
NW = 8                # n-windows of 256
NRC = 16              # r-chunks of 128

BF = ml_dtypes.bfloat16

_CACHE = {}


def _build_bass():
    import concourse.bass as bass
    import concourse.mybir as mybir
    import concourse.tile as tile
    from concourse import bacc

    dt = mybir.dt
    fp32 = dt.float32
    bf16 = dt.bfloat16
    EXP = mybir.ActivationFunctionType.Exp

    nc = bacc.Bacc("TRN2", target_bir_lowering=False, debug=False,
                   num_devices=NCORES)

    xT = nc.dram_tensor("xT", [DIM, N], bf16, kind="ExternalInput")
    wq = nc.dram_tensor("wq", [DIM, 256], bf16, kind="ExternalInput")
    wk = nc.dram_tensor("wk", [DIM, 256], bf16, kind="ExternalInput")
    wv = nc.dram_tensor("wv", [DIM, 256], bf16, kind="ExternalInput")
    relT = nc.dram_tensor("relT", [128, W], bf16, kind="ExternalInput")
    wo = nc.dram_tensor("wo", [256, DIM], bf16, kind="ExternalInput")
    edel = nc.dram_tensor("edel", [HPC, N], bf16, kind="ExternalInput")
    outT = nc.dram_tensor("outT", [DIM, N], fp32, kind="ExternalOutput")

    from contextlib import ExitStack
    with tile.TileContext(nc) as tc, ExitStack() as ctx:
        consts = ctx.enter_context(tc.tile_pool(name="consts", bufs=1))
        lpool = ctx.enter_context(tc.tile_pool(name="lpool", bufs=4))
        work = ctx.enter_context(tc.tile_pool(name="work", bufs=3))
        ppool = ctx.enter_context(tc.tile_pool(name="ppool", bufs=2))
        eppool = ctx.enter_context(tc.tile_pool(name="eppool", bufs=2))
        numep = ctx.enter_context(tc.tile_pool(name="numep", bufs=2))
        psb = ctx.enter_context(tc.tile_pool(name="psb", bufs=3, space="PSUM"))
        psa = ctx.enter_context(tc.tile_pool(name="psa", bufs=2, space="PSUM"))
        dramp = ctx.enter_context(tc.tile_pool(name="dramp", bufs=4, space="DRAM"))

        # ---- load constants (scalar-engine HWDGE queue) ---------------------
        xT_sb = consts.tile([128, 4, N], bf16, tag="xT_sb")
        nc.scalar.dma_start(out=xT_sb, in_=xT.ap().rearrange("(c p) n -> p c n", p=128))
        wq_sb = consts.tile([128, 4, 256], bf16, tag="wq_sb")
        nc.scalar.dma_start(out=wq_sb, in_=wq.ap().rearrange("(c p) i -> p c i", p=128))
        wk_sb = consts.tile([128, 4, 256], bf16, tag="wk_sb")
        nc.scalar.dma_start(out=wk_sb, in_=wk.ap().rearrange("(c p) i -> p c i", p=128))
        wv_sb = consts.tile([128, 4, 256], bf16, tag="wv_sb")
        nc.scalar.dma_start(out=wv_sb, in_=wv.ap().rearrange("(c p) i -> p c i", p=128))
        relT_sb = consts.tile([128, W], bf16, tag="relT_sb")
        nc.scalar.dma_start(out=relT_sb, in_=relT.ap())
        # wo packed as head-pairs: rows (hc*128 + hp*64 + d) -> [128, 2, DIM]
        wo_sb = consts.tile([128, 2, DIM], bf16, tag="wo_sb")
        nc.scalar.dma_start(out=wo_sb, in_=wo.ap().rearrange("(c p) o -> p c o", p=128))
        # edel replicated across all 128 partitions
        edel_sb = consts.tile([128, HPC, N], bf16, tag="edel_sb")
        edel_src = bass.AP(tensor=edel.ap().tensor, offset=edel.ap().offset,
                           ap=[[0, 128], [N, HPC], [1, N]])
        nc.scalar.dma_start(out=edel_sb, in_=edel_src)

        # ---- projections ----------------------------------------------------
        qT_sb = consts.tile([128, 2, N], bf16, tag="qT_sb")
        kT_sb = consts.tile([128, 2, N], bf16, tag="kT_sb")
        for dst_sb, w_sb in ((qT_sb, wq_sb), (kT_sb, wk_sb)):
            for ic in range(2):
                for nw in range(4):   # 512-wide windows
                    ps = psb.tile([128, 1024], fp32, tag="big")
                    for dc in range(4):
                        nc.tensor.matmul(
                            ps[:, 0:512],
                            lhsT=w_sb[:, dc, ic * 128:(ic + 1) * 128],
                            rhs=xT_sb[:, dc, nw * 512:(nw + 1) * 512],
                            start=(dc == 0), stop=(dc == 3))
                    nc.vector.tensor_copy(dst_sb[:, ic, nw * 512:(nw + 1) * 512],
                                          ps[:, 0:512])
        # v in [r, d-per-head] layout with a ones column per head: [128, rc, h, 65]
        v_sb = consts.tile([128, NRC, HPC, 65], bf16, tag="v_sb")
        nc.vector.memset(v_sb[:, :, :, 64], 1.0)
        for rc in range(NRC):
            ps = psb.tile([128, 1024], fp32, tag="big")
            for dc in range(4):
                nc.tensor.matmul(
                    ps[:, 0:256],
                    lhsT=xT_sb[:, dc, rc * 128:(rc + 1) * 128],
                    rhs=wv_sb[:, dc, :],
                    start=(dc == 0), stop=(dc == 3))
            nc.vector.tensor_copy(
                v_sb[:, rc, :, 0:64],
                ps[:, 0:256].rearrange("p (h d) -> p h d", h=HPC))

        # ---- per-head phases, emitted interleaved for pipelining ------------
        L_dram_h = [None] * HPC

        def emit_L(h):
            """q~ . relF logits for head h -> DRAM bf16 [N, W]."""
            hc, hp = h // 2, (h % 2) * 64
            L_dram = dramp.tile([N, W], bf16, tag="Ldram")
            L_dram_h[h] = L_dram
            for nck in range(NRC):
                lsb = lpool.tile([128, W], bf16, tag="lsb")
                psA = psb.tile([128, 1024], fp32, tag="big")
                for jw in range(2):
                    nc.tensor.matmul(
                        psA[:, jw * 512:(jw + 1) * 512],
                        lhsT=qT_sb[hp:hp + 64, hc, nck * 128:(nck + 1) * 128],
                        rhs=relT_sb[hp:hp + 64, jw * 512:(jw + 1) * 512],
                        start=True, stop=True)
                psB = psb.tile([128, 1024], fp32, tag="big")
                nc.tensor.matmul(
                    psB[:, 0:256],
                    lhsT=qT_sb[hp:hp + 64, hc, nck * 128:(nck + 1) * 128],
                    rhs=relT_sb[hp:hp + 64, 1024:1280],
                    start=True, stop=True)
                nc.vector.tensor_copy(lsb[:, 0:1024], psA)
                nc.vector.tensor_copy(lsb[:, 1024:1280], psB[:, 0:256])
                nc.gpsimd.dma_start(
                    out=L_dram[nck * 128:(nck + 1) * 128, :], in_=lsb)

        avn_all = consts.tile([128, 2, NW, 256], bf16, tag="avn_all")

        def emit_att(h):
            hc, hp = h // 2, (h % 2) * 64
            L_dram = L_dram_h[h]
            num_all = numep.tile([65, NW, 256], fp32, tag="num_all")
            den_all = numep.tile([8, 256], fp32, tag="den_all")
            recip_all = numep.tile([8, 256], fp32, tag="recip_all")
            recip_dram = dramp.tile([8, 256], fp32, tag="recip_dram")

            # merged skew transpose-DMAs: one per r-chunk, covering all its
            # band n-subchunks (diagonals d = rc - s in [-4, 4])
            ep_all = eppool.tile([128, NRC, 9, 128], bf16, tag="ep_all")
            for rc in range(NRC):
                s_lo, s_hi = max(0, rc - 4), min(NRC - 1, rc + 4)
                k = s_hi - s_lo + 1
                # src element (s, a, c) = L[128*s + c, PAD + 128*(rc-s) + 512 + a - c]
                off = (L_dram.offset + 128 * s_lo * W
                       + PAD_L + 128 * (rc - s_lo) + 512)
                src = bass.AP(tensor=L_dram.tensor, offset=off,
                              ap=[[W - 1, 128 * k], [1, 128]])
                slot0 = s_lo - (rc - 4)
                nc.sync.dma_start(out=ep_all[:, rc, slot0:slot0 + k, :],
                                  in_=src, transpose=True)

            for nw in range(NW):
                n0 = nw * 256
                s0 = 2 * nw                     # first n-sub of this window
                P_sb = ppool.tile([128, NRC, 256], bf16, tag="P_sb")
                # band-add cases (indices in global rc space):
                #   central run rc in [ca, cb]: slot pair (s0-rc+4, +1) as 256 cols
                #   rc = s0-4: first half slot 8 (second half left-clamp: +0)
                #   rc = s0+5: second half slot 0 (first half right-clamp: edel
                #              multiply post-exp)
                ca, cb = max(0, s0 - 3), min(NRC - 1, s0 + 4)
                for g in range(4):
                    ps = psb.tile([128, 1024], fp32, tag="big")
                    for i in range(4):
                        rc = 4 * g + i
                        nc.tensor.matmul(
                            ps[:, i * 256:(i + 1) * 256],
                            lhsT=kT_sb[hp:hp + 64, hc, rc * 128:(rc + 1) * 128],
                            rhs=qT_sb[hp:hp + 64, hc, n0:n0 + 256],
                            start=True, stop=True)
                    psv = ps.rearrange("p (i n) -> p i n", i=4)
                    # central-run portion inside this group
                    a, b = max(ca, 4 * g), min(cb, 4 * g + 3)
                    if b >= a:
                        kk = b - a + 1
                        epa = ep_all[:, a, s0 - a + 4, 0:128]   # first element
                        ep_run = bass.AP(
                            tensor=epa.tensor, offset=epa.offset,
                            ap=[list(epa.ap[0]), [8 * 128, kk], [1, 256]])
                        nc.vector.tensor_add(psv[:, a - 4 * g:b - 4 * g + 1, :],
                                             psv[:, a - 4 * g:b - 4 * g + 1, :],
                                             ep_run)
                    rc = s0 - 4
                    if 4 * g <= rc <= 4 * g + 3:
                        nc.vector.tensor_add(
                            ps[:, (rc - 4 * g) * 256:(rc - 4 * g) * 256 + 128],
                            ps[:, (rc - 4 * g) * 256:(rc - 4 * g) * 256 + 128],
                            ep_all[:, rc, 8, :])
                    rc = s0 + 5
                    if 4 * g <= rc <= 4 * g + 3 and rc <= NRC - 1:
                        nc.vector.tensor_add(
                            ps[:, (rc - 4 * g) * 256 + 128:(rc - 4 * g) * 256 + 256],
                            ps[:, (rc - 4 * g) * 256 + 128:(rc - 4 * g) * 256 + 256],
                            ep_all[:, rc, 0, :])
                    nc.scalar.activation(
                        P_sb[:, 4 * g:4 * (g + 1), :], psv, EXP)

                # post-exp right-clamp multiplies (edel factors)
                rc = s0 + 5
                if rc <= NRC - 1:
                    nc.vector.tensor_mul(
                        P_sb[:, rc, 0:128], P_sb[:, rc, 0:128],
                        edel_sb[:, h, n0:n0 + 128])
                rstart = s0 + 6
                if rstart < NRC:
                    k_right = NRC - rstart
                    ed = edel_sb[:, h, n0:n0 + 256]
                    ed_b = bass.AP(tensor=ed.tensor, offset=ed.offset,
                                   ap=[list(ed.ap[0]), [0, k_right], [1, 256]])
                    nc.vector.tensor_mul(
                        P_sb[:, rstart:NRC, :], P_sb[:, rstart:NRC, :], ed_b)

                # PV accumulate: acc[0:65, n] += [v|1].T @ P
                acc = psa.tile([128, 256], fp32, tag="acc")
                for rc in range(NRC):
                    nc.tensor.matmul(
                        acc[0:65, :],
                        lhsT=v_sb[:, rc, h, :],
                        rhs=P_sb[:, rc, :],
                        start=(rc == 0), stop=(rc == NRC - 1),
                        skip_group_check=True)
                nc.vector.tensor_copy(num_all[:, nw, :], acc[0:65, :])
                # collect denominator row for batched reciprocal
                nc.gpsimd.dma_start(out=den_all[nw:nw + 1, :],
                                    in_=num_all[64:65, nw, :])

            nc.vector.reciprocal(recip_all, den_all)
            nc.gpsimd.dma_start(out=recip_dram, in_=recip_all)
            for nw in range(NW):
                rb_bc = work.tile([64, 256], fp32, tag="rb_bc")
                rsrc = bass.AP(tensor=recip_dram.tensor,
                               offset=recip_dram.offset + nw * 256,
                               ap=[[0, 64], [1, 256]])
                nc.gpsimd.dma_start(out=rb_bc, in_=rsrc)
                if hp == 0:
                    nc.vector.tensor_mul(avn_all[0:64, hc, nw, :],
                                         num_all[0:64, nw, :], rb_bc)
                else:
                    # odd head of the pair lives on partitions 64-127; DVE
                    # can't shift partitions, so hop through a DMA
                    at = work.tile([64, 256], bf16, tag="avn_tmp")
                    nc.vector.tensor_mul(at, num_all[0:64, nw, :], rb_bc)
                    nc.gpsimd.dma_start(out=avn_all[64:128, hc, nw, :], in_=at)

        # pipeline heads: L(0), L(1), att(0), L(2), att(1), L(3), att(2), att(3)
        emit_L(0)
        emit_L(1)
        emit_att(0)
        emit_L(2)
        emit_att(1)
        emit_L(3)
        emit_att(2)
        emit_att(3)

        # ---- output projection (head-pair packed, 128-part matmuls) ---------
        for nw in range(NW):
            for oc in range(4):
                ps = psb.tile([128, 1024], fp32, tag="big")
                for hc in range(2):
                    nc.tensor.matmul(
                        ps[:, 0:256],
                        lhsT=wo_sb[:, hc, oc * 128:(oc + 1) * 128],
                        rhs=avn_all[:, hc, nw, :],
                        start=(hc == 0), stop=(hc == 1))
                o_sb = work.tile([128, 256], fp32, tag="o_sb")
                nc.vector.tensor_copy(o_sb, ps[:, 0:256])
                nc.gpsimd.dma_start(
                    out=outT.ap()[oc * 128:(oc + 1) * 128, nw * 256:(nw + 1) * 256],
                    in_=o_sb)

    nc.compile()
    return nc


def host_prep(x, Wq, Wkv, Wo, bo, rel_emb):
    """Build the 8 per-core input maps (all host-side prep is O(N*D))."""
    x = np.asarray(x, np.float32)
    Wq = np.asarray(Wq, np.float32)
    Wkv = np.asarray(Wkv, np.float32)
    Wo = np.asarray(Wo, np.float32)
    rel_emb = np.asarray(rel_emb, np.float32)

    # relF[j] = rel_emb[1024-j] - rel_emb[1024], edge-padded/clipped; [W, 64]
    jgrid = np.clip(np.arange(W) - PAD_L, 0, 1024)
    relF = rel_emb[1024 - jgrid] - rel_emb[1024]
    relT_one = np.ascontiguousarray(relF.T)            # [64, W]
    relT_in = np.concatenate([relT_one, relT_one], axis=0).astype(BF)  # [128, W]
    d_vec = rel_emb[0] - rel_emb[1024]                 # [64]

    in_maps = []
    for core in range(NCORES):
        b, hg = core // 2, core % 2
        sl = slice(hg * 256, (hg + 1) * 256)
        wq_s = (Wq[:, sl] * SCALE).astype(BF)
        # edelta per head: exp(x @ Wq~_h @ d_vec)
        delta = x[b] @ ((Wq[:, sl] * SCALE).reshape(DIM, HPC, D) @ d_vec)  # [N, HPC]
        in_maps.append({
            "xT": np.ascontiguousarray(x[b].T).astype(BF),
            "wq": wq_s,
            "wk": Wkv[:, sl].astype(BF),
            "wv": Wkv[:, 512 + hg * 256: 512 + (hg + 1) * 256].astype(BF),
            "relT": relT_in,
            "wo": Wo[sl, :].astype(BF),
            "edel": np.ascontiguousarray(np.exp(delta).T).astype(BF),  # [HPC, N]
        })
    return in_maps


def _install_ntff_hook():
    """The agent image's antenv lacks axon_hooks; synthesize it so
    run_bass_kernel_spmd(trace=True) can capture NTFF profiles."""
    import types
    try:
        if "antenv.axon_hooks" not in sys.modules:
            import antenv
            from trn_agent_boot.trn_boot import _ntff_profile_via_ctypes
            hooks = types.ModuleType("antenv.axon_hooks")
            state = {"h": _ntff_profile_via_ctypes("/opt/axon/libaxon_pjrt.so")}
            hooks.set_axon_ntff_profile_hook = lambda h: state.__setitem__("h", h)
            hooks.get_axon_ntff_profile_hook = lambda: state["h"]
            sys.modules["antenv.axon_hooks"] = hooks
            antenv.axon_hooks = hooks
        import antenv.axon_hooks as ah
        return ah.get_axon_ntff_profile_hook() is not None
    except Exception as e:
        print(f"ntff hook install failed: {e!r}")
        return False


def kernel(x, Wq, Wkv, Wo, bo, rel_emb, _trace=False):
    import concourse.bass_utils as bu
    from concourse.bass_utils import run_bass_kernel_spmd

    if "nc" not in _CACHE:
        _CACHE["nc"] = _build_bass()
    nc = _CACHE["nc"]

    in_maps = host_prep(x, Wq, Wkv, Wo, bo, rel_emb)
    kw = {}
    if _trace and _install_ntff_hook():
        bu.upload_artifacts = lambda d: d     # zero-egress: keep artifacts local
        tmpdir = "/root/problem/traces/latest"
        import shutil
        shutil.rmtree(tmpdir, ignore_errors=True)
        os.makedirs(tmpdir, exist_ok=True)
        kw = dict(trace=True, tmpdir=tmpdir)
    res = run_bass_kernel_spmd(nc, in_maps, list(range(NCORES)), **kw)
    _CACHE["last_result"] = res

    bo = np.asarray(bo, np.float32)
    out = np.empty((B, N, DIM), np.float32)
    for b in range(B):
        pT = res.results[2 * b]["outT"] + res.results[2 * b + 1]["outT"]
        out[b] = pT.T + bo[None, :]
    return out
